# revision 3
# baseline (speedup 1.0000x reference)
"""BreadthAttentionConv (GNN attention message passing) on 8 Trainium2 cores.

v2: dst-node partition (as baseline) with a restructured device pipeline.

Host-side layout: per core, nodes sorted by in-degree, grouped into blocks of
128 (SBUF partition dim). Incoming edges per node padded to the block's slot
count d_b (even). Blocks with d_b > CAP are split into chunks of <= CAP slots;
softmax is accumulated two-level (unnormalized numer/denom per chunk, combined
per block). The host ships h[src] in slot-column-major feat-on-partition
layout, so the device needs no gather.

Device, per chunk (d_c slots x 128 nodes), sub-batches of SUBG slots:
  pz[node, g*128 : g*128+64]  = scol_g.T @ WdT  + hpT_blk.T @ WsT   (z)
  pz[node, g*128+64 : +128]   = scol_g.T @ WmT                      (hm)
  t = tanh(z)            (ACT, psum->sbuf)
  tv = t * v             (DVE, in-place, 4x mode)
  e = reduce_add(tv)     (DVE, f32 out, 2x mode)
  e += mask - 3          (DVE; -3 bias bounds exp for fp16)
  p = exp(e)             (ACT, fp16)
  w = hm * p             (GPSIMD, psum->sbuf fused eviction+scale)
  numer += reduce_s(w)   (DVE, strided view [p, f, s], 2x mode)
  denom += reduce_s(p)   (DVE)
Per block: out = tanh(numer * (1/denom)) on ACT, grouped DMA out.
"""
import sys

for _p in ("/opt/trn_rl_repo",):
    if _p not in sys.path:
        sys.path.insert(0, _p)

import numpy as np

import concourse.bass as bass
import concourse.bacc as bacc
import concourse.tile as tile
from concourse import mybir
from concourse.bass_utils import run_bass_kernel_spmd

P = 128
NCORES = 8
MASK_VALID = -3.0   # softmax shift: keeps exp(e) in [e^-10, e^4] for fp16
MASK_PAD = -33.0
CAP = 32            # max slots per chunk (psum: 2 tiles of [128,16*128] f32)
SUBG = 16           # slot-columns per psum tile


# ---------------------------------------------------------------- host side
def _make_plan(deg_sorted_by_core):
    heads = deg_sorted_by_core[:, ::P]
    d = heads.max(axis=0)
    d = np.maximum(d, 1)
    d = ((d + 1) // 2) * 2
    return d.astype(np.int64)


def _make_chunks(d_blocks):
    """Split blocks into <=CAP-slot chunks: (node_block, col, d_c, first, last)."""
    chunks = []
    col = 0
    for b, db in enumerate(d_blocks):
        rem, first = int(db), True
        while rem > 0:
            dc = min(rem, CAP)
            rem -= dc
            chunks.append((b, col, dc, first, rem == 0))
            col += dc
            first = False
    return chunks


def _preprocess(h, edge_index, W_msg, Ws, Wd, v, ncores):
    n, in_dim = h.shape
    own = n // ncores
    n_blocks = (own + P - 1) // P
    own_pad = n_blocks * P

    ei = np.asarray(edge_index)
    loops = np.arange(n, dtype=ei.dtype)
    src = np.concatenate([ei[0], loops]).astype(np.int64)
    dst = np.concatenate([ei[1], loops]).astype(np.int64)

    deg = np.bincount(dst, minlength=n)
    core_of = dst // own

    perms = []
    deg_sorted = np.zeros((ncores, own_pad), dtype=np.int64)
    for c in range(ncores):
        d_c = deg[c * own : (c + 1) * own]
        perm = np.argsort(-d_c, kind="stable")
        perms.append(perm)
        deg_sorted[c, :own] = d_c[perm]
    d_blocks = _make_plan(deg_sorted)
    col_of_block = np.zeros(n_blocks + 1, dtype=np.int64)
    np.cumsum(d_blocks, out=col_of_block[1:])
    s_total = int(col_of_block[-1])
    chunks = _make_chunks(d_blocks)

    h32 = np.asarray(h, dtype=np.float32)
    h16 = h32.astype(np.float16)
    wdT = np.ascontiguousarray(np.asarray(Wd).T.astype(np.float16))   # [64,64]
    wsT = np.ascontiguousarray(np.asarray(Ws).T.astype(np.float16))
    wmT = np.ascontiguousarray(np.asarray(W_msg).T.astype(np.float16))
    # stack [WdT | WmT] -> one 128-col moving operand per src column
    wdm = np.ascontiguousarray(np.concatenate([wdT, wmT], axis=1))    # [64,128]
    vb = np.ascontiguousarray(np.tile(np.asarray(v).astype(np.float16), (P, 1)))

    in_maps = []
    for c in range(ncores):
        m = core_of == c
        src_c = src[m]
        dst_local = dst[m] - c * own
        perm = perms[c]
        rank = np.empty(own, dtype=np.int64)
        rank[perm] = np.arange(own)
        key = rank[dst_local]
        order = np.argsort(key, kind="stable")
        src_sorted = src_c[order]
        key_sorted = key[order]
        counts = np.bincount(key_sorted, minlength=own_pad)
        starts = np.zeros(own_pad + 1, dtype=np.int64)
        np.cumsum(counts, out=starts[1:])
        slot = np.arange(len(key_sorted)) - starts[key_sorted]
        blk = key_sorted // P
        part = key_sorted % P
        col = col_of_block[blk] * P + slot * P + part  # slot-column-major pos

        src_of_pos = np.zeros(s_total * P, dtype=np.int64)  # pad -> node 0
        src_of_pos[col] = src_sorted
        mask = np.full((P, s_total), MASK_PAD, dtype=np.float16)
        mask[part, col_of_block[blk] + slot] = MASK_VALID
        for r in range(own, own_pad):
            mask[r % P, col_of_block[r // P]] = MASK_VALID

        # h_srcT: [in_dim, s_total*128] fp16, column q holds h[src_of_pos[q]].
        # Packed chunk-major: chunk c occupies a contiguous 64*dc*128 block
        # (row stride dc*128 within the chunk) for DRAM-friendly DMA reads.
        h_srcT = h16[src_of_pos].T  # [64, s_total*128]
        packed = np.empty(64 * s_total * P, dtype=np.float16)
        pos = 0
        for _, coff, dcc, _, _ in chunks:
            blk = h_srcT[:, coff * P : (coff + dcc) * P]
            packed[pos : pos + blk.size] = blk.ravel()
            pos += blk.size
        h_srcT = packed.reshape(1, -1)
        hp = np.zeros((own_pad, in_dim), dtype=np.float16)
        hp[:own] = h16[c * own : (c + 1) * own][perm]
        hpT = np.ascontiguousarray(hp.T)
        in_maps.append(
            {
                "hsrcT": h_srcT,
                "hpT": hpT,
                "wdm": wdm,
                "wsT": wsT,
                "vb": vb,
                "mask": mask,
            }
        )
    meta = dict(
        n=n, own=own, own_pad=own_pad, n_blocks=n_blocks,
        d_blocks=d_blocks, chunks=chunks, perms=perms,
    )
    return in_maps, meta


# ---------------------------------------------------------------- device side
def _build_program(n_blocks, chunks, own_pad, in_dim=64, a_dim=64, out_dim=64):
    f16, f32 = mybir.dt.float16, mybir.dt.float32
    s_total = chunks[-1][1] + chunks[-1][2]

    nc = bacc.Bacc("TRN2", target_bir_lowering=False, debug=False)
    hsrcT = nc.dram_tensor(
        "hsrcT", [1, in_dim * s_total * P], f16, kind="ExternalInput"
    )
    hpT_d = nc.dram_tensor("hpT", [in_dim, own_pad], f16, kind="ExternalInput")
    wdm_d = nc.dram_tensor("wdm", [in_dim, 2 * a_dim], f16, kind="ExternalInput")
    wsT_d = nc.dram_tensor("wsT", [in_dim, a_dim], f16, kind="ExternalInput")
    vb_d = nc.dram_tensor("vb", [P, a_dim], f16, kind="ExternalInput")
    mask_d = nc.dram_tensor("mask", [P, s_total], f16, kind="ExternalInput")
    out_d = nc.dram_tensor(
        "out", [own_pad, out_dim], f32, kind="ExternalOutput"
    )

    with tile.TileContext(nc) as tc:
        with (
            tc.tile_pool(name="consts", bufs=1) as consts,
            tc.tile_pool(name="lhs", bufs=3) as lhs,
            tc.tile_pool(name="psum", bufs=2, space="PSUM") as psum,
            tc.tile_pool(name="work", bufs=3) as work,
            tc.tile_pool(name="small", bufs=4) as small,
            tc.tile_pool(name="acc", bufs=4) as accp,
            tc.tile_pool(name="outp", bufs=3) as outp,
        ):
            wdm_sb = consts.tile([in_dim, 2 * a_dim], f16)
            nc.sync.dma_start(out=wdm_sb[:], in_=wdm_d[:])
            wsT_sb = consts.tile([in_dim, a_dim], f16)
            nc.sync.dma_start(out=wsT_sb[:], in_=wsT_d[:])
            vb_sb = consts.tile([P, a_dim], f16)
            nc.sync.dma_start(out=vb_sb[:], in_=vb_d[:])
            mask_sb = consts.tile([P, s_total], f16)
            nc.sync.dma_start(out=mask_sb[:], in_=mask_d[:])
            hpT_sb = consts.tile([in_dim, own_pad], f16)
            nc.sync.dma_start(out=hpT_sb[:], in_=hpT_d[:])

            ob_group = 8
            out_t = None
            numer16 = None
            denom = None
            hsrc_off = 0
            for ci, (b, off, dc, first, last) in enumerate(chunks):
                ts = lhs.tile([in_dim, dc * P], f16, tag="ts")
                nc.sync.dma_start(
                    out=ts[:],
                    in_=bass.AP(
                        tensor=hsrcT,
                        offset=hsrc_off,
                        ap=[[dc * P, in_dim], [1, dc * P]],
                    ),
                )
                hsrc_off += in_dim * dc * P
                hp_b = hpT_sb[:, b * P : (b + 1) * P]

                t_sb = work.tile([P, dc * a_dim], f16, tag="t")
                tv_sb = work.tile([P, dc * a_dim], f16, tag="tv")
                hm_sb = work.tile([P, dc * out_dim], f16, tag="hm")
                w_sb = work.tile([P, dc * out_dim], f16, tag="w")
                e16 = small.tile([P, dc], f16, tag="e16")
                p_sb = small.tile([P, dc], f16, tag="p")
                t_v = t_sb[:].rearrange("p (g d) -> p g d", d=a_dim)
                hm_v = hm_sb[:].rearrange("p (g d) -> p g d", d=out_dim)

                n_sub = (dc + SUBG - 1) // SUBG
                for sb_i in range(n_sub):
                    g0 = sb_i * SUBG
                    gn = min(SUBG, dc - g0)
                    pz = psum.tile([P, SUBG * a_dim], f32, tag="pz")
                    ph = psum.tile([P, SUBG * out_dim], f32, tag="ph")
                    # z = Wd h_src + Ws h_dst ; hm = Wm h_src (per slot col)
                    for g in range(gn):
                        scol = ts[:, (g0 + g) * P : (g0 + g + 1) * P]
                        nc.tensor.matmul(
                            out=pz[:, g * a_dim : (g + 1) * a_dim],
                            lhsT=scol,
                            rhs=wdm_sb[:, :a_dim],
                            start=True,
                            stop=False,
                        )
                        nc.tensor.matmul(
                            out=pz[:, g * a_dim : (g + 1) * a_dim],
                            lhsT=hp_b,
                            rhs=wsT_sb[:],
                            start=False,
                            stop=True,
                        )
                        nc.tensor.matmul(
                            out=ph[:, g * out_dim : (g + 1) * out_dim],
                            lhsT=scol,
                            rhs=wdm_sb[:, a_dim:],
                            start=True,
                            stop=True,
                        )
                    # t = tanh(z)  (ACT, contiguous psum -> sbuf)
                    nc.scalar.activation(
                        out=t_sb[:, g0 * a_dim : (g0 + gn) * a_dim],
                        in_=pz[:, : gn * a_dim],
                        func=mybir.ActivationFunctionType.Tanh,
                    )
                    # evict hm psum -> sbuf fp16 (split DVE-heavy / ACT)
                    if True:
                        nc.scalar.activation(
                            out=hm_sb[:, g0 * out_dim : (g0 + gn) * out_dim],
                            in_=ph[:, : gn * out_dim],
                            func=mybir.ActivationFunctionType.Copy,
                        )
                    else:
                        nc.vector.tensor_scalar_mul(
                            out=hm_sb[:, g0 * out_dim : (g0 + gn) * out_dim],
                            in0=ph[:, : gn * out_dim],
                            scalar1=1.0,
                        )

                # tv = t * v (DVE, separate tile, whole chunk)
                t_c = t_v[:, :dc, :]
                tv_v = tv_sb[:].rearrange("p (g d) -> p g d", d=a_dim)
                nc.vector.tensor_tensor(
                    out=tv_v[:, :dc, :],
                    in0=t_c,
                    in1=vb_sb[:].unsqueeze(1).to_broadcast([P, dc, a_dim]),
                    op=mybir.AluOpType.mult,
                )
                # e = sum_f tv  (fp16 accumulate: max err ~4e-3 on e, ok)
                with nc.allow_low_precision("e in fp16: abs err <= 4e-3"):
                    nc.vector.tensor_reduce(
                        out=e16[:, :dc],
                        in_=tv_v[:, :dc, :],
                        axis=mybir.AxisListType.X,
                        op=mybir.AluOpType.add,
                    )
                # e += mask (-3 valid / -33 pad)
                nc.vector.tensor_tensor(
                    out=e16[:, :dc],
                    in0=e16[:, :dc],
                    in1=mask_sb[:, off : off + dc],
                    op=mybir.AluOpType.add,
                )
                # p = exp(e)
                nc.scalar.activation(
                    out=p_sb[:, :dc],
                    in_=e16[:, :dc],
                    func=mybir.ActivationFunctionType.Exp,
                )
                # w = hm * p  (GPSIMD, sbuf->sbuf, whole chunk)
                nc.gpsimd.tensor_tensor(
                    out=w_sb[:].rearrange("p (g d) -> p g d", d=out_dim),
                    in0=hm_v[:, :dc, :],
                    in1=p_sb[:, :dc]
                    .unsqueeze(2)
                    .to_broadcast([P, dc, out_dim]),
                    op=mybir.AluOpType.mult,
                )

                # numer_c = sum_s w: contiguous fold tree (stays in DVE 2x)
                gf = dc
                while gf > 2:
                    if gf % 2 == 1:
                        nc.vector.tensor_tensor(
                            out=w_sb[:, :out_dim],
                            in0=w_sb[:, :out_dim],
                            in1=w_sb[:, (gf - 1) * out_dim : gf * out_dim],
                            op=mybir.AluOpType.add,
                        )
                        gf -= 1
                        if gf == 2:
                            break
                    half = gf // 2
                    nc.vector.tensor_tensor(
                        out=w_sb[:, : half * out_dim],
                        in0=w_sb[:, : half * out_dim],
                        in1=w_sb[:, half * out_dim : 2 * half * out_dim],
                        op=mybir.AluOpType.add,
                    )
                    gf = half
                if first:
                    numer16 = accp.tile([P, out_dim], f16, tag="numer")
                    nc.vector.tensor_tensor(
                        out=numer16[:],
                        in0=w_sb[:, :out_dim],
                        in1=w_sb[:, out_dim : 2 * out_dim],
                        op=mybir.AluOpType.add,
                    )
                else:
                    nc.vector.tensor_tensor(
                        out=w_sb[:, :out_dim],
                        in0=w_sb[:, :out_dim],
                        in1=w_sb[:, out_dim : 2 * out_dim],
                        op=mybir.AluOpType.add,
                    )
                    nc.vector.tensor_tensor(
                        out=numer16[:], in0=numer16[:], in1=w_sb[:, :out_dim],
                        op=mybir.AluOpType.add,
                    )
                # denom_c = sum_s p
                if first:
                    denom = accp.tile([P, 1], f32, tag="denom")
                    nc.vector.tensor_reduce(
                        out=denom[:], in_=p_sb[:], axis=mybir.AxisListType.X,
                        op=mybir.AluOpType.add,
                    )
                else:
                    dtmp = small.tile([P, 1], f32, tag="dtmp")
                    nc.vector.tensor_reduce(
                        out=dtmp[:], in_=p_sb[:], axis=mybir.AxisListType.X,
                        op=mybir.AluOpType.add,
                    )
                    nc.vector.tensor_tensor(
                        out=denom[:], in0=denom[:], in1=dtmp[:],
                        op=mybir.AluOpType.add,
                    )

                if not last:
                    continue
                r_sb = small.tile([P, 1], f32, tag="r")
                nc.vector.reciprocal(out=r_sb[:], in_=denom[:])
                gi = b % ob_group
                if gi == 0:
                    out_t = outp.tile([P, ob_group * out_dim], f32, tag="ot")
                # out = tanh(numer * (1/denom)): the scale rides on ACT
                nc.scalar.activation(
                    out=out_t[:, gi * out_dim : (gi + 1) * out_dim],
                    in_=numer16[:],
                    func=mybir.ActivationFunctionType.Tanh,
                    scale=r_sb[:],
                )
                if gi == ob_group - 1 or b == n_blocks - 1:
                    ng = gi + 1
                    b0 = b - gi
                    nc.sync.dma_start(
                        out=bass.AP(
                            tensor=out_d,
                            offset=b0 * P * out_dim,
                            ap=[[out_dim, P], [P * out_dim, ng], [1, out_dim]],
                        ),
                        in_=out_t[:].rearrange("p (g d) -> p g d", d=out_dim)[
                            :, :ng, :
                        ],
                    )
    nc.compile()
    return nc


_CACHE = {}


def _get_program(meta):
    key = (
        meta["own_pad"], meta["n_blocks"],
        tuple((b, o, d) for b, o, d, _, _ in meta["chunks"]),
    )
    if key not in _CACHE:
        _CACHE[key] = _build_program(
            meta["n_blocks"], meta["chunks"], meta["own_pad"],
        )
    return _CACHE[key]


def run(h, edge_index, W_msg, Ws, Wd, v, trace=False, trace_kwargs=None):
    in_maps, meta = _preprocess(h, edge_index, W_msg, Ws, Wd, v, NCORES)
    nc = _get_program(meta)
    kwargs = {}
    if trace:
        kwargs = dict(trace=True, **(trace_kwargs or {}))
    res = run_bass_kernel_spmd(nc, in_maps, list(range(NCORES)), **kwargs)
    n, own = meta["n"], meta["own"]
    out_dim = res.results[0]["out"].shape[1]
    full = np.zeros((n, out_dim), dtype=np.float32)
    for c in range(NCORES):
        perm = meta["perms"][c]
        full[c * own + perm] = res.results[c]["out"][:own]
    return full, res


def _spot_check(out, h, edge_index, W_msg, Ws, Wd, v, k=128):
    """Exact fp32 reference on k sampled dst nodes; guards against the rare
    corrupted device execution (re-run once if it trips)."""
    h = np.asarray(h, np.float64)
    ei = np.asarray(edge_index)
    n = h.shape[0]
    loops = np.arange(n, dtype=ei.dtype)
    src = np.concatenate([ei[0], loops])
    dst = np.concatenate([ei[1], loops])
    order = np.argsort(dst, kind="stable")
    dst_s, src_s = dst[order], src[order]
    rng = np.random.default_rng(12345)
    nodes = rng.choice(n, size=k, replace=False)
    lo = np.searchsorted(dst_s, nodes, side="left")
    hi = np.searchsorted(dst_s, nodes, side="right")
    Wsm, Wdm, Wmm = (np.asarray(W, np.float64) for W in (Ws, Wd, W_msg))
    vv = np.asarray(v, np.float64)
    bad = 0
    for j, node in enumerate(nodes):
        sj = src_s[lo[j] : hi[j]]
        e = np.tanh(h[node] @ Wsm.T + h[sj] @ Wdm.T) @ vv
        ex = np.exp(e - e.max())
        alpha = ex / ex.sum()
        ref = np.tanh(alpha @ (h[sj] @ Wmm.T))
        if np.abs(ref - out[node]).max() > 0.05:
            bad += 1
    return bad == 0


def kernel(h, edge_index, W_msg, Ws, Wd, v):
    out, _ = run(h, edge_index, W_msg, Ws, Wd, v)
    if not _spot_check(out, h, edge_index, W_msg, Ws, Wd, v):
        out, _ = run(h, edge_index, W_msg, Ws, Wd, v)
    return out


# revision 4
# speedup vs baseline: 1.0125x; 1.0125x over previous
"""BreadthAttentionConv (GNN attention message passing) on 8 Trainium2 cores.

v2: dst-node partition (as baseline) with a restructured device pipeline.

Host-side layout: per core, nodes sorted by in-degree, grouped into blocks of
128 (SBUF partition dim). Incoming edges per node padded to the block's slot
count d_b (even). Blocks with d_b > CAP are split into chunks of <= CAP slots;
softmax is accumulated two-level (unnormalized numer/denom per chunk, combined
per block). The host ships h[src] in slot-column-major feat-on-partition
layout, so the device needs no gather.

Device, per chunk (d_c slots x 128 nodes), sub-batches of SUBG slots:
  pz[node, g*128 : g*128+64]  = scol_g.T @ WdT  + hpT_blk.T @ WsT   (z)
  pz[node, g*128+64 : +128]   = scol_g.T @ WmT                      (hm)
  t = tanh(z)            (ACT, psum->sbuf)
  tv = t * v             (DVE, in-place, 4x mode)
  e = reduce_add(tv)     (DVE, f32 out, 2x mode)
  e += mask - 3          (DVE; -3 bias bounds exp for fp16)
  p = exp(e)             (ACT, fp16)
  w = hm * p             (GPSIMD, psum->sbuf fused eviction+scale)
  numer += reduce_s(w)   (DVE, strided view [p, f, s], 2x mode)
  denom += reduce_s(p)   (DVE)
Per block: out = tanh(numer * (1/denom)) on ACT, grouped DMA out.
"""
import sys

for _p in ("/opt/trn_rl_repo",):
    if _p not in sys.path:
        sys.path.insert(0, _p)

import numpy as np

import concourse.bass as bass
import concourse.bacc as bacc
import concourse.tile as tile
from concourse import mybir
from concourse.bass_utils import run_bass_kernel_spmd

P = 128
NCORES = 8
MASK_VALID = -3.0   # softmax shift: keeps exp(e) in [e^-10, e^4] for fp16
MASK_PAD = -33.0
CAP = 64            # no block splitting needed (evict is per-sub, unscaled)
SUBG = 16           # slot-columns per psum tile


# ---------------------------------------------------------------- host side
def _make_plan(deg_sorted_by_core):
    heads = deg_sorted_by_core[:, ::P]
    d = heads.max(axis=0)
    d = np.maximum(d, 1)
    d = ((d + 1) // 2) * 2
    return d.astype(np.int64)


def _make_chunks(d_blocks):
    """Split blocks into <=CAP-slot chunks: (node_block, col, d_c, first, last)."""
    chunks = []
    col = 0
    for b, db in enumerate(d_blocks):
        rem, first = int(db), True
        while rem > 0:
            dc = min(rem, CAP)
            rem -= dc
            chunks.append((b, col, dc, first, rem == 0))
            col += dc
            first = False
    return chunks


def _preprocess(h, edge_index, W_msg, Ws, Wd, v, ncores):
    n, in_dim = h.shape
    own = n // ncores
    n_blocks = (own + P - 1) // P
    own_pad = n_blocks * P

    ei = np.asarray(edge_index)
    loops = np.arange(n, dtype=ei.dtype)
    src = np.concatenate([ei[0], loops]).astype(np.int64)
    dst = np.concatenate([ei[1], loops]).astype(np.int64)

    deg = np.bincount(dst, minlength=n)
    core_of = dst // own

    perms = []
    deg_sorted = np.zeros((ncores, own_pad), dtype=np.int64)
    for c in range(ncores):
        d_c = deg[c * own : (c + 1) * own]
        perm = np.argsort(-d_c, kind="stable")
        perms.append(perm)
        deg_sorted[c, :own] = d_c[perm]
    d_blocks = _make_plan(deg_sorted)
    col_of_block = np.zeros(n_blocks + 1, dtype=np.int64)
    np.cumsum(d_blocks, out=col_of_block[1:])
    s_total = int(col_of_block[-1])
    chunks = _make_chunks(d_blocks)

    h32 = np.asarray(h, dtype=np.float32)
    h16 = h32.astype(np.float16)
    wdT = np.ascontiguousarray(np.asarray(Wd).T.astype(np.float16))   # [64,64]
    wsT = np.ascontiguousarray(np.asarray(Ws).T.astype(np.float16))
    wmT = np.ascontiguousarray(np.asarray(W_msg).T.astype(np.float16))
    # stack [WdT | WmT] -> one 128-col moving operand per src column
    wdm = np.ascontiguousarray(np.concatenate([wdT, wmT], axis=1))    # [64,128]
    vb = np.ascontiguousarray(np.tile(np.asarray(v).astype(np.float16), (P, 1)))

    in_maps = []
    for c in range(ncores):
        m = core_of == c
        src_c = src[m]
        dst_local = dst[m] - c * own
        perm = perms[c]
        rank = np.empty(own, dtype=np.int64)
        rank[perm] = np.arange(own)
        key = rank[dst_local]
        order = np.argsort(key, kind="stable")
        src_sorted = src_c[order]
        key_sorted = key[order]
        counts = np.bincount(key_sorted, minlength=own_pad)
        starts = np.zeros(own_pad + 1, dtype=np.int64)
        np.cumsum(counts, out=starts[1:])
        slot = np.arange(len(key_sorted)) - starts[key_sorted]
        blk = key_sorted // P
        part = key_sorted % P
        col = col_of_block[blk] * P + slot * P + part  # slot-column-major pos

        src_of_pos = np.zeros(s_total * P, dtype=np.int64)  # pad -> node 0
        src_of_pos[col] = src_sorted
        mask = np.full((P, s_total), MASK_PAD, dtype=np.float16)
        mask[part, col_of_block[blk] + slot] = MASK_VALID
        for r in range(own, own_pad):
            mask[r % P, col_of_block[r // P]] = MASK_VALID

        # h_srcT: [in_dim, s_total*128] fp16, column q holds h[src_of_pos[q]].
        # Packed chunk-major: chunk c occupies a contiguous 64*dc*128 block
        # (row stride dc*128 within the chunk) for DRAM-friendly DMA reads.
        h_srcT = h16[src_of_pos].T  # [64, s_total*128]
        packed = np.empty(64 * s_total * P, dtype=np.float16)
        pos = 0
        for _, coff, dcc, _, _ in chunks:
            blk = h_srcT[:, coff * P : (coff + dcc) * P]
            packed[pos : pos + blk.size] = blk.ravel()
            pos += blk.size
        h_srcT = packed.reshape(1, -1)
        hp = np.zeros((own_pad, in_dim), dtype=np.float16)
        hp[:own] = h16[c * own : (c + 1) * own][perm]
        hpT = np.ascontiguousarray(hp.T)
        in_maps.append(
            {
                "hsrcT": h_srcT,
                "hpT": hpT,
                "wdm": wdm,
                "wsT": wsT,
                "vb": vb,
                "mask": mask,
            }
        )
    meta = dict(
        n=n, own=own, own_pad=own_pad, n_blocks=n_blocks,
        d_blocks=d_blocks, chunks=chunks, perms=perms,
    )
    return in_maps, meta


# ---------------------------------------------------------------- device side
def _build_program(n_blocks, chunks, own_pad, in_dim=64, a_dim=64, out_dim=64):
    f16, f32 = mybir.dt.float16, mybir.dt.float32
    s_total = chunks[-1][1] + chunks[-1][2]

    nc = bacc.Bacc("TRN2", target_bir_lowering=False, debug=False)
    hsrcT = nc.dram_tensor(
        "hsrcT", [1, in_dim * s_total * P], f16, kind="ExternalInput"
    )
    hpT_d = nc.dram_tensor("hpT", [in_dim, own_pad], f16, kind="ExternalInput")
    wdm_d = nc.dram_tensor("wdm", [in_dim, 2 * a_dim], f16, kind="ExternalInput")
    wsT_d = nc.dram_tensor("wsT", [in_dim, a_dim], f16, kind="ExternalInput")
    vb_d = nc.dram_tensor("vb", [P, a_dim], f16, kind="ExternalInput")
    mask_d = nc.dram_tensor("mask", [P, s_total], f16, kind="ExternalInput")
    out_d = nc.dram_tensor(
        "out", [own_pad, out_dim], f32, kind="ExternalOutput"
    )

    with tile.TileContext(nc) as tc:
        with (
            tc.tile_pool(name="consts", bufs=1) as consts,
            tc.tile_pool(name="lhs", bufs=3) as lhs,
            tc.tile_pool(name="psum", bufs=2, space="PSUM") as psum,
            tc.tile_pool(name="work", bufs=3) as work,
            tc.tile_pool(name="small", bufs=4) as small,
            tc.tile_pool(name="acc", bufs=4) as accp,
            tc.tile_pool(name="outp", bufs=3) as outp,
        ):
            wdm_sb = consts.tile([in_dim, 2 * a_dim], f16)
            nc.sync.dma_start(out=wdm_sb[:], in_=wdm_d[:])
            wsT_sb = consts.tile([in_dim, a_dim], f16)
            nc.sync.dma_start(out=wsT_sb[:], in_=wsT_d[:])
            vb_sb = consts.tile([P, a_dim], f16)
            nc.sync.dma_start(out=vb_sb[:], in_=vb_d[:])
            mask_sb = consts.tile([P, s_total], f16)
            nc.sync.dma_start(out=mask_sb[:], in_=mask_d[:])
            hpT_sb = consts.tile([in_dim, own_pad], f16)
            nc.sync.dma_start(out=hpT_sb[:], in_=hpT_d[:])

            ob_group = 8
            out_t = None
            numer16 = None
            denom = None
            hsrc_off = 0
            for ci, (b, off, dc, first, last) in enumerate(chunks):
                ts = lhs.tile([in_dim, dc * P], f16, tag="ts")
                nc.sync.dma_start(
                    out=ts[:],
                    in_=bass.AP(
                        tensor=hsrcT,
                        offset=hsrc_off,
                        ap=[[dc * P, in_dim], [1, dc * P]],
                    ),
                )
                hsrc_off += in_dim * dc * P
                hp_b = hpT_sb[:, b * P : (b + 1) * P]

                t_sb = work.tile([P, dc * a_dim], f16, tag="t")
                tv_sb = work.tile([P, dc * a_dim], f16, tag="tv")
                hm_sb = work.tile([P, dc * out_dim], f16, tag="hm")
                w_sb = work.tile([P, dc * out_dim], f16, tag="w")
                e16 = small.tile([P, dc], f16, tag="e16")
                p_sb = small.tile([P, dc], f16, tag="p")
                t_v = t_sb[:].rearrange("p (g d) -> p g d", d=a_dim)
                hm_v = hm_sb[:].rearrange("p (g d) -> p g d", d=out_dim)

                n_sub = (dc + SUBG - 1) // SUBG
                for sb_i in range(n_sub):
                    g0 = sb_i * SUBG
                    gn = min(SUBG, dc - g0)
                    pz = psum.tile([P, SUBG * a_dim], f32, tag="pz")
                    ph = psum.tile([P, SUBG * out_dim], f32, tag="ph")
                    # z = Wd h_src + Ws h_dst ; hm = Wm h_src (per slot col)
                    for g in range(gn):
                        scol = ts[:, (g0 + g) * P : (g0 + g + 1) * P]
                        nc.tensor.matmul(
                            out=pz[:, g * a_dim : (g + 1) * a_dim],
                            lhsT=scol,
                            rhs=wdm_sb[:, :a_dim],
                            start=True,
                            stop=False,
                        )
                        nc.tensor.matmul(
                            out=pz[:, g * a_dim : (g + 1) * a_dim],
                            lhsT=hp_b,
                            rhs=wsT_sb[:],
                            start=False,
                            stop=True,
                        )
                        nc.tensor.matmul(
                            out=ph[:, g * out_dim : (g + 1) * out_dim],
                            lhsT=scol,
                            rhs=wdm_sb[:, a_dim:],
                            start=True,
                            stop=True,
                        )
                    # t = tanh(z)  (ACT, contiguous psum -> sbuf)
                    nc.scalar.activation(
                        out=t_sb[:, g0 * a_dim : (g0 + gn) * a_dim],
                        in_=pz[:, : gn * a_dim],
                        func=mybir.ActivationFunctionType.Tanh,
                    )
                    # evict hm psum -> sbuf fp16 (split DVE-heavy / ACT)
                    if True:
                        nc.scalar.activation(
                            out=hm_sb[:, g0 * out_dim : (g0 + gn) * out_dim],
                            in_=ph[:, : gn * out_dim],
                            func=mybir.ActivationFunctionType.Copy,
                        )
                    else:
                        nc.vector.tensor_scalar_mul(
                            out=hm_sb[:, g0 * out_dim : (g0 + gn) * out_dim],
                            in0=ph[:, : gn * out_dim],
                            scalar1=1.0,
                        )

                # tv = t * v (DVE, separate tile, whole chunk)
                t_c = t_v[:, :dc, :]
                tv_v = tv_sb[:].rearrange("p (g d) -> p g d", d=a_dim)
                nc.vector.tensor_tensor(
                    out=tv_v[:, :dc, :],
                    in0=t_c,
                    in1=vb_sb[:].unsqueeze(1).to_broadcast([P, dc, a_dim]),
                    op=mybir.AluOpType.mult,
                )
                # e = sum_f tv  (fp16 accumulate: max err ~4e-3 on e, ok)
                with nc.allow_low_precision("e in fp16: abs err <= 4e-3"):
                    nc.vector.tensor_reduce(
                        out=e16[:, :dc],
                        in_=tv_v[:, :dc, :],
                        axis=mybir.AxisListType.X,
                        op=mybir.AluOpType.add,
                    )
                # e += mask (-3 valid / -33 pad)
                nc.vector.tensor_tensor(
                    out=e16[:, :dc],
                    in0=e16[:, :dc],
                    in1=mask_sb[:, off : off + dc],
                    op=mybir.AluOpType.add,
                )
                # p = exp(e)
                nc.scalar.activation(
                    out=p_sb[:, :dc],
                    in_=e16[:, :dc],
                    func=mybir.ActivationFunctionType.Exp,
                )
                # w = hm * p  (GPSIMD, sbuf->sbuf, whole chunk)
                nc.gpsimd.tensor_tensor(
                    out=w_sb[:].rearrange("p (g d) -> p g d", d=out_dim),
                    in0=hm_v[:, :dc, :],
                    in1=p_sb[:, :dc]
                    .unsqueeze(2)
                    .to_broadcast([P, dc, out_dim]),
                    op=mybir.AluOpType.mult,
                )

                # numer_c = sum_s w: contiguous fold tree (stays in DVE 2x)
                gf = dc
                while gf > 2:
                    if gf % 2 == 1:
                        nc.vector.tensor_tensor(
                            out=w_sb[:, :out_dim],
                            in0=w_sb[:, :out_dim],
                            in1=w_sb[:, (gf - 1) * out_dim : gf * out_dim],
                            op=mybir.AluOpType.add,
                        )
                        gf -= 1
                        if gf == 2:
                            break
                    half = gf // 2
                    nc.vector.tensor_tensor(
                        out=w_sb[:, : half * out_dim],
                        in0=w_sb[:, : half * out_dim],
                        in1=w_sb[:, half * out_dim : 2 * half * out_dim],
                        op=mybir.AluOpType.add,
                    )
                    gf = half
                if first:
                    numer16 = accp.tile([P, out_dim], f16, tag="numer")
                    nc.vector.tensor_tensor(
                        out=numer16[:],
                        in0=w_sb[:, :out_dim],
                        in1=w_sb[:, out_dim : 2 * out_dim],
                        op=mybir.AluOpType.add,
                    )
                else:
                    nc.vector.tensor_tensor(
                        out=w_sb[:, :out_dim],
                        in0=w_sb[:, :out_dim],
                        in1=w_sb[:, out_dim : 2 * out_dim],
                        op=mybir.AluOpType.add,
                    )
                    nc.vector.tensor_tensor(
                        out=numer16[:], in0=numer16[:], in1=w_sb[:, :out_dim],
                        op=mybir.AluOpType.add,
                    )
                # denom_c = sum_s p
                if first:
                    denom = accp.tile([P, 1], f32, tag="denom")
                    nc.vector.tensor_reduce(
                        out=denom[:], in_=p_sb[:], axis=mybir.AxisListType.X,
                        op=mybir.AluOpType.add,
                    )
                else:
                    dtmp = small.tile([P, 1], f32, tag="dtmp")
                    nc.vector.tensor_reduce(
                        out=dtmp[:], in_=p_sb[:], axis=mybir.AxisListType.X,
                        op=mybir.AluOpType.add,
                    )
                    nc.vector.tensor_tensor(
                        out=denom[:], in0=denom[:], in1=dtmp[:],
                        op=mybir.AluOpType.add,
                    )

                if not last:
                    continue
                r_sb = small.tile([P, 1], f32, tag="r")
                nc.vector.reciprocal(out=r_sb[:], in_=denom[:])
                gi = b % ob_group
                if gi == 0:
                    out_t = outp.tile([P, ob_group * out_dim], f32, tag="ot")
                # out = tanh(numer * (1/denom)): the scale rides on ACT
                nc.scalar.activation(
                    out=out_t[:, gi * out_dim : (gi + 1) * out_dim],
                    in_=numer16[:],
                    func=mybir.ActivationFunctionType.Tanh,
                    scale=r_sb[:],
                )
                if gi == ob_group - 1 or b == n_blocks - 1:
                    ng = gi + 1
                    b0 = b - gi
                    nc.sync.dma_start(
                        out=bass.AP(
                            tensor=out_d,
                            offset=b0 * P * out_dim,
                            ap=[[out_dim, P], [P * out_dim, ng], [1, out_dim]],
                        ),
                        in_=out_t[:].rearrange("p (g d) -> p g d", d=out_dim)[
                            :, :ng, :
                        ],
                    )
    nc.compile()
    return nc


_CACHE = {}


def _get_program(meta):
    key = (
        meta["own_pad"], meta["n_blocks"],
        tuple((b, o, d) for b, o, d, _, _ in meta["chunks"]),
    )
    if key not in _CACHE:
        _CACHE[key] = _build_program(
            meta["n_blocks"], meta["chunks"], meta["own_pad"],
        )
    return _CACHE[key]


def run(h, edge_index, W_msg, Ws, Wd, v, trace=False, trace_kwargs=None):
    in_maps, meta = _preprocess(h, edge_index, W_msg, Ws, Wd, v, NCORES)
    nc = _get_program(meta)
    kwargs = {}
    if trace:
        kwargs = dict(trace=True, **(trace_kwargs or {}))
    res = run_bass_kernel_spmd(nc, in_maps, list(range(NCORES)), **kwargs)
    n, own = meta["n"], meta["own"]
    out_dim = res.results[0]["out"].shape[1]
    full = np.zeros((n, out_dim), dtype=np.float32)
    for c in range(NCORES):
        perm = meta["perms"][c]
        full[c * own + perm] = res.results[c]["out"][:own]
    return full, res


def _spot_check(out, h, edge_index, W_msg, Ws, Wd, v, k=128):
    """Exact fp32 reference on k sampled dst nodes; guards against the rare
    corrupted device execution (re-run once if it trips)."""
    h = np.asarray(h, np.float64)
    ei = np.asarray(edge_index)
    n = h.shape[0]
    loops = np.arange(n, dtype=ei.dtype)
    src = np.concatenate([ei[0], loops])
    dst = np.concatenate([ei[1], loops])
    order = np.argsort(dst, kind="stable")
    dst_s, src_s = dst[order], src[order]
    rng = np.random.default_rng(12345)
    nodes = rng.choice(n, size=k, replace=False)
    lo = np.searchsorted(dst_s, nodes, side="left")
    hi = np.searchsorted(dst_s, nodes, side="right")
    Wsm, Wdm, Wmm = (np.asarray(W, np.float64) for W in (Ws, Wd, W_msg))
    vv = np.asarray(v, np.float64)
    bad = 0
    for j, node in enumerate(nodes):
        sj = src_s[lo[j] : hi[j]]
        e = np.tanh(h[node] @ Wsm.T + h[sj] @ Wdm.T) @ vv
        ex = np.exp(e - e.max())
        alpha = ex / ex.sum()
        ref = np.tanh(alpha @ (h[sj] @ Wmm.T))
        if np.abs(ref - out[node]).max() > 0.05:
            bad += 1
    return bad == 0


def kernel(h, edge_index, W_msg, Ws, Wd, v):
    out, _ = run(h, edge_index, W_msg, Ws, Wd, v)
    if not _spot_check(out, h, edge_index, W_msg, Ws, Wd, v):
        out, _ = run(h, edge_index, W_msg, Ws, Wd, v)
    return out


# revision 5
# speedup vs baseline: 1.0152x; 1.0026x over previous
"""BreadthAttentionConv (GNN attention message passing) on 8 Trainium2 cores.

v2: dst-node partition (as baseline) with a restructured device pipeline.

Host-side layout: per core, nodes sorted by in-degree, grouped into blocks of
128 (SBUF partition dim). Incoming edges per node padded to the block's slot
count d_b (even). Blocks with d_b > CAP are split into chunks of <= CAP slots;
softmax is accumulated two-level (unnormalized numer/denom per chunk, combined
per block). The host ships h[src] in slot-column-major feat-on-partition
layout, so the device needs no gather.

Device, per chunk (d_c slots x 128 nodes), sub-batches of SUBG slots:
  pz[node, g*128 : g*128+64]  = scol_g.T @ WdT  + hpT_blk.T @ WsT   (z)
  pz[node, g*128+64 : +128]   = scol_g.T @ WmT                      (hm)
  t = tanh(z)            (ACT, psum->sbuf)
  tv = t * v             (DVE, in-place, 4x mode)
  e = reduce_add(tv)     (DVE, f32 out, 2x mode)
  e += mask - 3          (DVE; -3 bias bounds exp for fp16)
  p = exp(e)             (ACT, fp16)
  w = hm * p             (GPSIMD, psum->sbuf fused eviction+scale)
  numer += reduce_s(w)   (DVE, strided view [p, f, s], 2x mode)
  denom += reduce_s(p)   (DVE)
Per block: out = tanh(numer * (1/denom)) on ACT, grouped DMA out.
"""
import sys

for _p in ("/opt/trn_rl_repo",):
    if _p not in sys.path:
        sys.path.insert(0, _p)

import numpy as np

import concourse.bass as bass
import concourse.bacc as bacc
import concourse.tile as tile
from concourse import mybir
from concourse.bass_utils import run_bass_kernel_spmd

P = 128
NCORES = 8
MASK_VALID = -3.0   # softmax shift: keeps exp(e) in [e^-10, e^4] for fp16
MASK_PAD = -33.0
CAP = 64            # no block splitting needed (evict is per-sub, unscaled)
SUBG = 16           # slot-columns per psum tile


# ---------------------------------------------------------------- host side
def _make_plan(deg_sorted_by_core):
    heads = deg_sorted_by_core[:, ::P]
    d = heads.max(axis=0)
    d = np.maximum(d, 1)
    d = ((d + 1) // 2) * 2
    return d.astype(np.int64)


def _make_chunks(d_blocks):
    """Split blocks into <=CAP-slot chunks: (node_block, col, d_c, first, last)."""
    chunks = []
    col = 0
    for b, db in enumerate(d_blocks):
        rem, first = int(db), True
        while rem > 0:
            dc = min(rem, CAP)
            rem -= dc
            chunks.append((b, col, dc, first, rem == 0))
            col += dc
            first = False
    return chunks


def _preprocess(h, edge_index, W_msg, Ws, Wd, v, ncores):
    n, in_dim = h.shape
    own = n // ncores
    n_blocks = (own + P - 1) // P
    own_pad = n_blocks * P

    ei = np.asarray(edge_index)
    loops = np.arange(n, dtype=ei.dtype)
    src = np.concatenate([ei[0], loops]).astype(np.int64)
    dst = np.concatenate([ei[1], loops]).astype(np.int64)

    deg = np.bincount(dst, minlength=n)
    core_of = dst // own

    perms = []
    deg_sorted = np.zeros((ncores, own_pad), dtype=np.int64)
    for c in range(ncores):
        d_c = deg[c * own : (c + 1) * own]
        perm = np.argsort(-d_c, kind="stable")
        perms.append(perm)
        deg_sorted[c, :own] = d_c[perm]
    d_blocks = _make_plan(deg_sorted)
    col_of_block = np.zeros(n_blocks + 1, dtype=np.int64)
    np.cumsum(d_blocks, out=col_of_block[1:])
    s_total = int(col_of_block[-1])
    chunks = _make_chunks(d_blocks)

    h32 = np.asarray(h, dtype=np.float32)
    h16 = h32.astype(np.float16)
    wdT = np.ascontiguousarray(np.asarray(Wd).T.astype(np.float16))   # [64,64]
    wsT = np.ascontiguousarray(np.asarray(Ws).T.astype(np.float16))
    wmT = np.ascontiguousarray(np.asarray(W_msg).T.astype(np.float16))
    # stack [WdT | WmT] -> one 128-col moving operand per src column
    wdm = np.ascontiguousarray(np.concatenate([wdT, wmT], axis=1))    # [64,128]
    vb = np.ascontiguousarray(np.tile(np.asarray(v).astype(np.float16), (P, 1)))

    in_maps = []
    for c in range(ncores):
        m = core_of == c
        src_c = src[m]
        dst_local = dst[m] - c * own
        perm = perms[c]
        rank = np.empty(own, dtype=np.int64)
        rank[perm] = np.arange(own)
        key = rank[dst_local]
        order = np.argsort(key, kind="stable")
        src_sorted = src_c[order]
        key_sorted = key[order]
        counts = np.bincount(key_sorted, minlength=own_pad)
        starts = np.zeros(own_pad + 1, dtype=np.int64)
        np.cumsum(counts, out=starts[1:])
        slot = np.arange(len(key_sorted)) - starts[key_sorted]
        blk = key_sorted // P
        part = key_sorted % P
        col = col_of_block[blk] * P + slot * P + part  # slot-column-major pos

        src_of_pos = np.zeros(s_total * P, dtype=np.int64)  # pad -> node 0
        src_of_pos[col] = src_sorted
        mask = np.full((P, s_total), MASK_PAD, dtype=np.float16)
        mask[part, col_of_block[blk] + slot] = MASK_VALID
        for r in range(own, own_pad):
            mask[r % P, col_of_block[r // P]] = MASK_VALID

        # h_srcT: [in_dim, s_total*128] fp16, column q holds h[src_of_pos[q]].
        # Packed chunk-major: chunk c occupies a contiguous 64*dc*128 block
        # (row stride dc*128 within the chunk) for DRAM-friendly DMA reads.
        h_srcT = h16[src_of_pos].T  # [64, s_total*128]
        packed = np.empty(64 * s_total * P, dtype=np.float16)
        pos = 0
        for _, coff, dcc, _, _ in chunks:
            blk = h_srcT[:, coff * P : (coff + dcc) * P]
            packed[pos : pos + blk.size] = blk.ravel()
            pos += blk.size
        h_srcT = packed.reshape(1, -1)
        hp = np.zeros((own_pad, in_dim), dtype=np.float16)
        hp[:own] = h16[c * own : (c + 1) * own][perm]
        hpT = np.ascontiguousarray(hp.T)
        in_maps.append(
            {
                "hsrcT": h_srcT,
                "hpT": hpT,
                "wdm": wdm,
                "wsT": wsT,
                "vb": vb,
                "mask": mask,
            }
        )
    meta = dict(
        n=n, own=own, own_pad=own_pad, n_blocks=n_blocks,
        d_blocks=d_blocks, chunks=chunks, perms=perms,
    )
    return in_maps, meta


# ---------------------------------------------------------------- device side
def _build_program(n_blocks, chunks, own_pad, in_dim=64, a_dim=64, out_dim=64):
    f16, f32 = mybir.dt.float16, mybir.dt.float32
    s_total = chunks[-1][1] + chunks[-1][2]

    nc = bacc.Bacc("TRN2", target_bir_lowering=False, debug=False)
    hsrcT = nc.dram_tensor(
        "hsrcT", [1, in_dim * s_total * P], f16, kind="ExternalInput"
    )
    hpT_d = nc.dram_tensor("hpT", [in_dim, own_pad], f16, kind="ExternalInput")
    wdm_d = nc.dram_tensor("wdm", [in_dim, 2 * a_dim], f16, kind="ExternalInput")
    wsT_d = nc.dram_tensor("wsT", [in_dim, a_dim], f16, kind="ExternalInput")
    vb_d = nc.dram_tensor("vb", [P, a_dim], f16, kind="ExternalInput")
    mask_d = nc.dram_tensor("mask", [P, s_total], f16, kind="ExternalInput")
    out_d = nc.dram_tensor(
        "out", [own_pad, out_dim], f32, kind="ExternalOutput"
    )

    with tile.TileContext(nc) as tc:
        with (
            tc.tile_pool(name="consts", bufs=1) as consts,
            tc.tile_pool(name="lhs", bufs=3) as lhs,
            tc.tile_pool(name="psum", bufs=2, space="PSUM") as psum,
            tc.tile_pool(name="work", bufs=3) as work,
            tc.tile_pool(name="small", bufs=4) as small,
            tc.tile_pool(name="acc", bufs=4) as accp,
            tc.tile_pool(name="outp", bufs=3) as outp,
        ):
            wdm_sb = consts.tile([in_dim, 2 * a_dim], f16)
            nc.sync.dma_start(out=wdm_sb[:], in_=wdm_d[:])
            wsT_sb = consts.tile([in_dim, a_dim], f16)
            nc.sync.dma_start(out=wsT_sb[:], in_=wsT_d[:])
            vb_sb = consts.tile([P, a_dim], f16)
            nc.sync.dma_start(out=vb_sb[:], in_=vb_d[:])
            mask_sb = consts.tile([P, s_total], f16)
            nc.sync.dma_start(out=mask_sb[:], in_=mask_d[:])


            ob_group = 8
            out_t = None
            numer16 = None
            denom = None
            hsrc_off = 0
            for ci, (b, off, dc, first, last) in enumerate(chunks):
                ts = lhs.tile([in_dim, dc * P], f16, tag="ts")
                nc.sync.dma_start(
                    out=ts[:],
                    in_=bass.AP(
                        tensor=hsrcT,
                        offset=hsrc_off,
                        ap=[[dc * P, in_dim], [1, dc * P]],
                    ),
                )
                hsrc_off += in_dim * dc * P
                hp_b_t = consts.tile([in_dim, P], f16, tag=f"hp{b}")
                nc.sync.dma_start(
                    out=hp_b_t[:], in_=hpT_d[:, b * P : (b + 1) * P]
                )
                hp_b = hp_b_t[:]

                t_sb = work.tile([P, dc * a_dim], f16, tag="t")
                tv_sb = work.tile([P, dc * a_dim], f16, tag="tv")
                hm_sb = work.tile([P, dc * out_dim], f16, tag="hm")
                w_sb = work.tile([P, dc * out_dim], f16, tag="w")
                e16 = small.tile([P, dc], f16, tag="e16")
                p_sb = small.tile([P, dc], f16, tag="p")
                t_v = t_sb[:].rearrange("p (g d) -> p g d", d=a_dim)
                hm_v = hm_sb[:].rearrange("p (g d) -> p g d", d=out_dim)

                n_sub = (dc + SUBG - 1) // SUBG
                for sb_i in range(n_sub):
                    g0 = sb_i * SUBG
                    gn = min(SUBG, dc - g0)
                    pz = psum.tile([P, SUBG * a_dim], f32, tag="pz")
                    ph = psum.tile([P, SUBG * out_dim], f32, tag="ph")
                    # z = Wd h_src + Ws h_dst ; hm = Wm h_src (per slot col)
                    for g in range(gn):
                        scol = ts[:, (g0 + g) * P : (g0 + g + 1) * P]
                        nc.tensor.matmul(
                            out=pz[:, g * a_dim : (g + 1) * a_dim],
                            lhsT=scol,
                            rhs=wdm_sb[:, :a_dim],
                            start=True,
                            stop=False,
                        )
                        nc.tensor.matmul(
                            out=pz[:, g * a_dim : (g + 1) * a_dim],
                            lhsT=hp_b,
                            rhs=wsT_sb[:],
                            start=False,
                            stop=True,
                        )
                        nc.tensor.matmul(
                            out=ph[:, g * out_dim : (g + 1) * out_dim],
                            lhsT=scol,
                            rhs=wdm_sb[:, a_dim:],
                            start=True,
                            stop=True,
                        )
                    # t = tanh(z)  (ACT, contiguous psum -> sbuf)
                    nc.scalar.activation(
                        out=t_sb[:, g0 * a_dim : (g0 + gn) * a_dim],
                        in_=pz[:, : gn * a_dim],
                        func=mybir.ActivationFunctionType.Tanh,
                    )
                    # evict hm psum -> sbuf fp16 (split DVE-heavy / ACT)
                    if True:
                        nc.scalar.activation(
                            out=hm_sb[:, g0 * out_dim : (g0 + gn) * out_dim],
                            in_=ph[:, : gn * out_dim],
                            func=mybir.ActivationFunctionType.Copy,
                        )
                    else:
                        nc.vector.tensor_scalar_mul(
                            out=hm_sb[:, g0 * out_dim : (g0 + gn) * out_dim],
                            in0=ph[:, : gn * out_dim],
                            scalar1=1.0,
                        )

                # tv = t * v (DVE, separate tile, whole chunk)
                t_c = t_v[:, :dc, :]
                tv_v = tv_sb[:].rearrange("p (g d) -> p g d", d=a_dim)
                nc.vector.tensor_tensor(
                    out=tv_v[:, :dc, :],
                    in0=t_c,
                    in1=vb_sb[:].unsqueeze(1).to_broadcast([P, dc, a_dim]),
                    op=mybir.AluOpType.mult,
                )
                # e = sum_f tv  (fp16 accumulate: max err ~4e-3 on e, ok)
                with nc.allow_low_precision("e in fp16: abs err <= 4e-3"):
                    nc.vector.tensor_reduce(
                        out=e16[:, :dc],
                        in_=tv_v[:, :dc, :],
                        axis=mybir.AxisListType.X,
                        op=mybir.AluOpType.add,
                    )
                # e += mask (-3 valid / -33 pad)
                nc.vector.tensor_tensor(
                    out=e16[:, :dc],
                    in0=e16[:, :dc],
                    in1=mask_sb[:, off : off + dc],
                    op=mybir.AluOpType.add,
                )
                # p = exp(e)
                nc.scalar.activation(
                    out=p_sb[:, :dc],
                    in_=e16[:, :dc],
                    func=mybir.ActivationFunctionType.Exp,
                )
                # w = hm * p  (GPSIMD, sbuf->sbuf, whole chunk)
                nc.gpsimd.tensor_tensor(
                    out=w_sb[:].rearrange("p (g d) -> p g d", d=out_dim),
                    in0=hm_v[:, :dc, :],
                    in1=p_sb[:, :dc]
                    .unsqueeze(2)
                    .to_broadcast([P, dc, out_dim]),
                    op=mybir.AluOpType.mult,
                )

                # numer_c = sum_s w: contiguous fold tree (stays in DVE 2x)
                gf = dc
                while gf > 2:
                    if gf % 2 == 1:
                        nc.vector.tensor_tensor(
                            out=w_sb[:, :out_dim],
                            in0=w_sb[:, :out_dim],
                            in1=w_sb[:, (gf - 1) * out_dim : gf * out_dim],
                            op=mybir.AluOpType.add,
                        )
                        gf -= 1
                        if gf == 2:
                            break
                    half = gf // 2
                    nc.vector.tensor_tensor(
                        out=w_sb[:, : half * out_dim],
                        in0=w_sb[:, : half * out_dim],
                        in1=w_sb[:, half * out_dim : 2 * half * out_dim],
                        op=mybir.AluOpType.add,
                    )
                    gf = half
                if first:
                    numer16 = accp.tile([P, out_dim], f16, tag="numer")
                    nc.vector.tensor_tensor(
                        out=numer16[:],
                        in0=w_sb[:, :out_dim],
                        in1=w_sb[:, out_dim : 2 * out_dim],
                        op=mybir.AluOpType.add,
                    )
                else:
                    nc.vector.tensor_tensor(
                        out=w_sb[:, :out_dim],
                        in0=w_sb[:, :out_dim],
                        in1=w_sb[:, out_dim : 2 * out_dim],
                        op=mybir.AluOpType.add,
                    )
                    nc.vector.tensor_tensor(
                        out=numer16[:], in0=numer16[:], in1=w_sb[:, :out_dim],
                        op=mybir.AluOpType.add,
                    )
                # denom_c = sum_s p
                if first:
                    denom = accp.tile([P, 1], f32, tag="denom")
                    nc.vector.tensor_reduce(
                        out=denom[:], in_=p_sb[:], axis=mybir.AxisListType.X,
                        op=mybir.AluOpType.add,
                    )
                else:
                    dtmp = small.tile([P, 1], f32, tag="dtmp")
                    nc.vector.tensor_reduce(
                        out=dtmp[:], in_=p_sb[:], axis=mybir.AxisListType.X,
                        op=mybir.AluOpType.add,
                    )
                    nc.vector.tensor_tensor(
                        out=denom[:], in0=denom[:], in1=dtmp[:],
                        op=mybir.AluOpType.add,
                    )

                if not last:
                    continue
                r_sb = small.tile([P, 1], f32, tag="r")
                nc.vector.reciprocal(out=r_sb[:], in_=denom[:])
                gi = b % ob_group
                if gi == 0:
                    out_t = outp.tile([P, ob_group * out_dim], f32, tag="ot")
                # out = tanh(numer * (1/denom)): the scale rides on ACT
                nc.scalar.activation(
                    out=out_t[:, gi * out_dim : (gi + 1) * out_dim],
                    in_=numer16[:],
                    func=mybir.ActivationFunctionType.Tanh,
                    scale=r_sb[:],
                )
                if gi == ob_group - 1 or b == n_blocks - 1:
                    ng = gi + 1
                    b0 = b - gi
                    nc.sync.dma_start(
                        out=bass.AP(
                            tensor=out_d,
                            offset=b0 * P * out_dim,
                            ap=[[out_dim, P], [P * out_dim, ng], [1, out_dim]],
                        ),
                        in_=out_t[:].rearrange("p (g d) -> p g d", d=out_dim)[
                            :, :ng, :
                        ],
                    )
    nc.compile()
    return nc


_CACHE = {}


def _get_program(meta):
    key = (
        meta["own_pad"], meta["n_blocks"],
        tuple((b, o, d) for b, o, d, _, _ in meta["chunks"]),
    )
    if key not in _CACHE:
        _CACHE[key] = _build_program(
            meta["n_blocks"], meta["chunks"], meta["own_pad"],
        )
    return _CACHE[key]


def run(h, edge_index, W_msg, Ws, Wd, v, trace=False, trace_kwargs=None):
    in_maps, meta = _preprocess(h, edge_index, W_msg, Ws, Wd, v, NCORES)
    nc = _get_program(meta)
    kwargs = {}
    if trace:
        kwargs = dict(trace=True, **(trace_kwargs or {}))
    res = run_bass_kernel_spmd(nc, in_maps, list(range(NCORES)), **kwargs)
    n, own = meta["n"], meta["own"]
    out_dim = res.results[0]["out"].shape[1]
    full = np.zeros((n, out_dim), dtype=np.float32)
    for c in range(NCORES):
        perm = meta["perms"][c]
        full[c * own + perm] = res.results[c]["out"][:own]
    return full, res


def _spot_check(out, h, edge_index, W_msg, Ws, Wd, v, k=128):
    """Exact fp32 reference on k sampled dst nodes; guards against the rare
    corrupted device execution (re-run once if it trips)."""
    h = np.asarray(h, np.float64)
    ei = np.asarray(edge_index)
    n = h.shape[0]
    loops = np.arange(n, dtype=ei.dtype)
    src = np.concatenate([ei[0], loops])
    dst = np.concatenate([ei[1], loops])
    order = np.argsort(dst, kind="stable")
    dst_s, src_s = dst[order], src[order]
    rng = np.random.default_rng(12345)
    nodes = rng.choice(n, size=k, replace=False)
    lo = np.searchsorted(dst_s, nodes, side="left")
    hi = np.searchsorted(dst_s, nodes, side="right")
    Wsm, Wdm, Wmm = (np.asarray(W, np.float64) for W in (Ws, Wd, W_msg))
    vv = np.asarray(v, np.float64)
    bad = 0
    for j, node in enumerate(nodes):
        sj = src_s[lo[j] : hi[j]]
        e = np.tanh(h[node] @ Wsm.T + h[sj] @ Wdm.T) @ vv
        ex = np.exp(e - e.max())
        alpha = ex / ex.sum()
        ref = np.tanh(alpha @ (h[sj] @ Wmm.T))
        if np.abs(ref - out[node]).max() > 0.05:
            bad += 1
    return bad == 0


def kernel(h, edge_index, W_msg, Ws, Wd, v):
    out, _ = run(h, edge_index, W_msg, Ws, Wd, v)
    if not _spot_check(out, h, edge_index, W_msg, Ws, Wd, v):
        out, _ = run(h, edge_index, W_msg, Ws, Wd, v)
    return out


# revision 6
# speedup vs baseline: 1.0214x; 1.0061x over previous
"""BreadthAttentionConv (GNN attention message passing) on 8 Trainium2 cores.

v2: dst-node partition (as baseline) with a restructured device pipeline.

Host-side layout: per core, nodes sorted by in-degree, grouped into blocks of
128 (SBUF partition dim). Incoming edges per node padded to the block's slot
count d_b (even). Blocks with d_b > CAP are split into chunks of <= CAP slots;
softmax is accumulated two-level (unnormalized numer/denom per chunk, combined
per block). The host ships h[src] in slot-column-major feat-on-partition
layout, so the device needs no gather.

Device, per chunk (d_c slots x 128 nodes), sub-batches of SUBG slots:
  pz[node, g*128 : g*128+64]  = scol_g.T @ WdT  + hpT_blk.T @ WsT   (z)
  pz[node, g*128+64 : +128]   = scol_g.T @ WmT                      (hm)
  t = tanh(z)            (ACT, psum->sbuf)
  tv = t * v             (DVE, in-place, 4x mode)
  e = reduce_add(tv)     (DVE, f32 out, 2x mode)
  e += mask - 3          (DVE; -3 bias bounds exp for fp16)
  p = exp(e)             (ACT, fp16)
  w = hm * p             (GPSIMD, psum->sbuf fused eviction+scale)
  numer += reduce_s(w)   (DVE, strided view [p, f, s], 2x mode)
  denom += reduce_s(p)   (DVE)
Per block: out = tanh(numer * (1/denom)) on ACT, grouped DMA out.
"""
import sys

for _p in ("/opt/trn_rl_repo",):
    if _p not in sys.path:
        sys.path.insert(0, _p)

import numpy as np

import concourse.bass as bass
import concourse.bacc as bacc
import concourse.tile as tile
from concourse import mybir
from concourse.bass_utils import run_bass_kernel_spmd

P = 128
NCORES = 8
MASK_VALID = -3.0   # softmax shift: keeps exp(e) in [e^-10, e^4] for fp16
MASK_PAD = -33.0
CAP = 64            # no block splitting needed (evict is per-sub, unscaled)
SUBG = 16           # slot-columns per psum tile


# ---------------------------------------------------------------- host side
def _make_plan(deg_sorted_by_core):
    heads = deg_sorted_by_core[:, ::P]
    d = heads.max(axis=0)
    d = np.maximum(d, 1)
    d = ((d + 1) // 2) * 2
    return d.astype(np.int64)


def _make_chunks(d_blocks):
    """Split blocks into <=CAP-slot chunks: (node_block, col, d_c, first, last)."""
    chunks = []
    col = 0
    for b, db in enumerate(d_blocks):
        rem, first = int(db), True
        while rem > 0:
            dc = min(rem, CAP)
            rem -= dc
            chunks.append((b, col, dc, first, rem == 0))
            col += dc
            first = False
    return chunks


def _preprocess(h, edge_index, W_msg, Ws, Wd, v, ncores):
    n, in_dim = h.shape
    own = n // ncores
    n_blocks = (own + P - 1) // P
    own_pad = n_blocks * P

    ei = np.asarray(edge_index)
    loops = np.arange(n, dtype=ei.dtype)
    src = np.concatenate([ei[0], loops]).astype(np.int64)
    dst = np.concatenate([ei[1], loops]).astype(np.int64)

    deg = np.bincount(dst, minlength=n)
    core_of = dst // own

    perms = []
    deg_sorted = np.zeros((ncores, own_pad), dtype=np.int64)
    for c in range(ncores):
        d_c = deg[c * own : (c + 1) * own]
        perm = np.argsort(-d_c, kind="stable")
        perms.append(perm)
        deg_sorted[c, :own] = d_c[perm]
    d_blocks = _make_plan(deg_sorted)
    col_of_block = np.zeros(n_blocks + 1, dtype=np.int64)
    np.cumsum(d_blocks, out=col_of_block[1:])
    s_total = int(col_of_block[-1])
    chunks = _make_chunks(d_blocks)

    h32 = np.asarray(h, dtype=np.float32)
    h16 = h32.astype(np.float16)
    wdT = np.ascontiguousarray(np.asarray(Wd).T.astype(np.float16))   # [64,64]
    wsT = np.ascontiguousarray(np.asarray(Ws).T.astype(np.float16))
    wmT = np.ascontiguousarray(np.asarray(W_msg).T.astype(np.float16))
    # stack [WdT | WmT] -> one 128-col moving operand per src column
    wdm = np.ascontiguousarray(np.concatenate([wdT, wmT], axis=1))    # [64,128]
    vb = np.ascontiguousarray(np.tile(np.asarray(v).astype(np.float16), (P, 1)))

    in_maps = []
    for c in range(ncores):
        m = core_of == c
        src_c = src[m]
        dst_local = dst[m] - c * own
        perm = perms[c]
        rank = np.empty(own, dtype=np.int64)
        rank[perm] = np.arange(own)
        key = rank[dst_local]
        order = np.argsort(key, kind="stable")
        src_sorted = src_c[order]
        key_sorted = key[order]
        counts = np.bincount(key_sorted, minlength=own_pad)
        starts = np.zeros(own_pad + 1, dtype=np.int64)
        np.cumsum(counts, out=starts[1:])
        slot = np.arange(len(key_sorted)) - starts[key_sorted]
        blk = key_sorted // P
        part = key_sorted % P
        col = col_of_block[blk] * P + slot * P + part  # slot-column-major pos

        src_of_pos = np.zeros(s_total * P, dtype=np.int64)  # pad -> node 0
        src_of_pos[col] = src_sorted
        mask = np.full((P, s_total), MASK_PAD, dtype=np.float16)
        mask[part, col_of_block[blk] + slot] = MASK_VALID
        for r in range(own, own_pad):
            mask[r % P, col_of_block[r // P]] = MASK_VALID

        # h_srcT: [in_dim, s_total*128] fp16, column q holds h[src_of_pos[q]].
        # Packed chunk-major: chunk c occupies a contiguous 64*dc*128 block
        # (row stride dc*128 within the chunk) for DRAM-friendly DMA reads.
        h_srcT = h16[src_of_pos].T  # [64, s_total*128]
        packed = np.empty(64 * s_total * P, dtype=np.float16)
        pos = 0
        for _, coff, dcc, _, _ in chunks:
            blk = h_srcT[:, coff * P : (coff + dcc) * P]
            packed[pos : pos + blk.size] = blk.ravel()
            pos += blk.size
        h_srcT = packed.reshape(1, -1)
        hp = np.zeros((own_pad, in_dim), dtype=np.float16)
        hp[:own] = h16[c * own : (c + 1) * own][perm]
        hpT = np.ascontiguousarray(hp.T)
        in_maps.append(
            {
                "hsrcT": h_srcT,
                "hpT": hpT,
                "wdm": wdm,
                "wsT": wsT,
                "vb": vb,
                "mask": mask,
            }
        )
    meta = dict(
        n=n, own=own, own_pad=own_pad, n_blocks=n_blocks,
        d_blocks=d_blocks, chunks=chunks, perms=perms,
    )
    return in_maps, meta


# ---------------------------------------------------------------- device side
def _build_program(n_blocks, chunks, own_pad, in_dim=64, a_dim=64, out_dim=64):
    f16, f32 = mybir.dt.float16, mybir.dt.float32
    s_total = chunks[-1][1] + chunks[-1][2]

    nc = bacc.Bacc("TRN2", target_bir_lowering=False, debug=False)
    hsrcT = nc.dram_tensor(
        "hsrcT", [1, in_dim * s_total * P], f16, kind="ExternalInput"
    )
    hpT_d = nc.dram_tensor("hpT", [in_dim, own_pad], f16, kind="ExternalInput")
    wdm_d = nc.dram_tensor("wdm", [in_dim, 2 * a_dim], f16, kind="ExternalInput")
    wsT_d = nc.dram_tensor("wsT", [in_dim, a_dim], f16, kind="ExternalInput")
    vb_d = nc.dram_tensor("vb", [P, a_dim], f16, kind="ExternalInput")
    mask_d = nc.dram_tensor("mask", [P, s_total], f16, kind="ExternalInput")
    out_d = nc.dram_tensor(
        "out", [own_pad, out_dim], f32, kind="ExternalOutput"
    )

    with tile.TileContext(nc) as tc:
        with (
            tc.tile_pool(name="consts", bufs=1) as consts,
            tc.tile_pool(name="lhs", bufs=3) as lhs,
            tc.tile_pool(name="psum", bufs=2, space="PSUM") as psum,
            tc.tile_pool(name="work", bufs=3) as work,
            tc.tile_pool(name="small", bufs=4) as small,
            tc.tile_pool(name="acc", bufs=4) as accp,
            tc.tile_pool(name="outp", bufs=3) as outp,
        ):
            wdm_sb = consts.tile([in_dim, 2 * a_dim], f16)
            nc.sync.dma_start(out=wdm_sb[:], in_=wdm_d[:])
            wsT_sb = consts.tile([in_dim, a_dim], f16)
            nc.sync.dma_start(out=wsT_sb[:], in_=wsT_d[:])
            vb_sb = consts.tile([P, a_dim], f16)
            nc.sync.dma_start(out=vb_sb[:], in_=vb_d[:])



            ob_group = 8
            out_t = None
            numer16 = None
            denom = None
            hsrc_off = 0
            for ci, (b, off, dc, first, last) in enumerate(chunks):
                ts = lhs.tile([in_dim, dc * P], f16, tag="ts")
                nc.sync.dma_start(
                    out=ts[:],
                    in_=bass.AP(
                        tensor=hsrcT,
                        offset=hsrc_off,
                        ap=[[dc * P, in_dim], [1, dc * P]],
                    ),
                )
                hsrc_off += in_dim * dc * P
                mk_t = consts.tile([P, CAP], f16, tag=f"mk{ci}")
                nc.sync.dma_start(
                    out=mk_t[:, :dc], in_=mask_d[:, off : off + dc]
                )
                hp_b_t = consts.tile([in_dim, P], f16, tag=f"hp{b}")
                nc.sync.dma_start(
                    out=hp_b_t[:], in_=hpT_d[:, b * P : (b + 1) * P]
                )
                hp_b = hp_b_t[:]

                t_sb = work.tile([P, dc * a_dim], f16, tag="t")
                tv_sb = work.tile([P, dc * a_dim], f16, tag="tv")
                hm_sb = work.tile([P, dc * out_dim], f16, tag="hm")
                w_sb = work.tile([P, dc * out_dim], f16, tag="w")
                e16 = small.tile([P, dc], f16, tag="e16")
                p_sb = small.tile([P, dc], f16, tag="p")
                t_v = t_sb[:].rearrange("p (g d) -> p g d", d=a_dim)
                hm_v = hm_sb[:].rearrange("p (g d) -> p g d", d=out_dim)

                n_sub = (dc + SUBG - 1) // SUBG
                for sb_i in range(n_sub):
                    g0 = sb_i * SUBG
                    gn = min(SUBG, dc - g0)
                    pz = psum.tile([P, SUBG * a_dim], f32, tag="pz")
                    ph = psum.tile([P, SUBG * out_dim], f32, tag="ph")
                    # z = Wd h_src + Ws h_dst ; hm = Wm h_src (per slot col)
                    for g in range(gn):
                        scol = ts[:, (g0 + g) * P : (g0 + g + 1) * P]
                        nc.tensor.matmul(
                            out=pz[:, g * a_dim : (g + 1) * a_dim],
                            lhsT=scol,
                            rhs=wdm_sb[:, :a_dim],
                            start=True,
                            stop=False,
                        )
                        nc.tensor.matmul(
                            out=pz[:, g * a_dim : (g + 1) * a_dim],
                            lhsT=hp_b,
                            rhs=wsT_sb[:],
                            start=False,
                            stop=True,
                        )
                        nc.tensor.matmul(
                            out=ph[:, g * out_dim : (g + 1) * out_dim],
                            lhsT=scol,
                            rhs=wdm_sb[:, a_dim:],
                            start=True,
                            stop=True,
                        )
                    # t = tanh(z)  (ACT, contiguous psum -> sbuf)
                    nc.scalar.activation(
                        out=t_sb[:, g0 * a_dim : (g0 + gn) * a_dim],
                        in_=pz[:, : gn * a_dim],
                        func=mybir.ActivationFunctionType.Tanh,
                    )
                    # evict hm psum -> sbuf fp16 (split DVE-heavy / ACT)
                    if True:
                        nc.scalar.activation(
                            out=hm_sb[:, g0 * out_dim : (g0 + gn) * out_dim],
                            in_=ph[:, : gn * out_dim],
                            func=mybir.ActivationFunctionType.Copy,
                        )
                    else:
                        nc.vector.tensor_scalar_mul(
                            out=hm_sb[:, g0 * out_dim : (g0 + gn) * out_dim],
                            in0=ph[:, : gn * out_dim],
                            scalar1=1.0,
                        )

                # tv = t * v (DVE, separate tile, whole chunk)
                t_c = t_v[:, :dc, :]
                tv_v = tv_sb[:].rearrange("p (g d) -> p g d", d=a_dim)
                nc.vector.tensor_tensor(
                    out=tv_v[:, :dc, :],
                    in0=t_c,
                    in1=vb_sb[:].unsqueeze(1).to_broadcast([P, dc, a_dim]),
                    op=mybir.AluOpType.mult,
                )
                # e = sum_f tv  (fp16 accumulate: max err ~4e-3 on e, ok)
                with nc.allow_low_precision("e in fp16: abs err <= 4e-3"):
                    nc.vector.tensor_reduce(
                        out=e16[:, :dc],
                        in_=tv_v[:, :dc, :],
                        axis=mybir.AxisListType.X,
                        op=mybir.AluOpType.add,
                    )
                # e += mask (-3 valid / -33 pad)
                nc.vector.tensor_tensor(
                    out=e16[:, :dc],
                    in0=e16[:, :dc],
                    in1=mk_t[:, :dc],
                    op=mybir.AluOpType.add,
                )
                # p = exp(e)
                nc.scalar.activation(
                    out=p_sb[:, :dc],
                    in_=e16[:, :dc],
                    func=mybir.ActivationFunctionType.Exp,
                )
                # w = hm * p  (GPSIMD, sbuf->sbuf, whole chunk)
                nc.gpsimd.tensor_tensor(
                    out=w_sb[:].rearrange("p (g d) -> p g d", d=out_dim),
                    in0=hm_v[:, :dc, :],
                    in1=p_sb[:, :dc]
                    .unsqueeze(2)
                    .to_broadcast([P, dc, out_dim]),
                    op=mybir.AluOpType.mult,
                )

                # numer_c = sum_s w: contiguous fold tree (stays in DVE 2x)
                gf = dc
                while gf > 2:
                    if gf % 2 == 1:
                        nc.vector.tensor_tensor(
                            out=w_sb[:, :out_dim],
                            in0=w_sb[:, :out_dim],
                            in1=w_sb[:, (gf - 1) * out_dim : gf * out_dim],
                            op=mybir.AluOpType.add,
                        )
                        gf -= 1
                        if gf == 2:
                            break
                    half = gf // 2
                    nc.vector.tensor_tensor(
                        out=w_sb[:, : half * out_dim],
                        in0=w_sb[:, : half * out_dim],
                        in1=w_sb[:, half * out_dim : 2 * half * out_dim],
                        op=mybir.AluOpType.add,
                    )
                    gf = half
                if first:
                    numer16 = accp.tile([P, out_dim], f16, tag="numer")
                    nc.vector.tensor_tensor(
                        out=numer16[:],
                        in0=w_sb[:, :out_dim],
                        in1=w_sb[:, out_dim : 2 * out_dim],
                        op=mybir.AluOpType.add,
                    )
                else:
                    nc.vector.tensor_tensor(
                        out=w_sb[:, :out_dim],
                        in0=w_sb[:, :out_dim],
                        in1=w_sb[:, out_dim : 2 * out_dim],
                        op=mybir.AluOpType.add,
                    )
                    nc.vector.tensor_tensor(
                        out=numer16[:], in0=numer16[:], in1=w_sb[:, :out_dim],
                        op=mybir.AluOpType.add,
                    )
                # denom_c = sum_s p
                if first:
                    denom = accp.tile([P, 1], f32, tag="denom")
                    nc.vector.tensor_reduce(
                        out=denom[:], in_=p_sb[:], axis=mybir.AxisListType.X,
                        op=mybir.AluOpType.add,
                    )
                else:
                    dtmp = small.tile([P, 1], f32, tag="dtmp")
                    nc.vector.tensor_reduce(
                        out=dtmp[:], in_=p_sb[:], axis=mybir.AxisListType.X,
                        op=mybir.AluOpType.add,
                    )
                    nc.vector.tensor_tensor(
                        out=denom[:], in0=denom[:], in1=dtmp[:],
                        op=mybir.AluOpType.add,
                    )

                if not last:
                    continue
                r_sb = small.tile([P, 1], f32, tag="r")
                nc.vector.reciprocal(out=r_sb[:], in_=denom[:])
                gi = b % ob_group
                if gi == 0:
                    out_t = outp.tile([P, ob_group * out_dim], f32, tag="ot")
                # out = tanh(numer * (1/denom)): the scale rides on ACT
                nc.scalar.activation(
                    out=out_t[:, gi * out_dim : (gi + 1) * out_dim],
                    in_=numer16[:],
                    func=mybir.ActivationFunctionType.Tanh,
                    scale=r_sb[:],
                )
                if gi == ob_group - 1 or b == n_blocks - 1:
                    ng = gi + 1
                    b0 = b - gi
                    nc.sync.dma_start(
                        out=bass.AP(
                            tensor=out_d,
                            offset=b0 * P * out_dim,
                            ap=[[out_dim, P], [P * out_dim, ng], [1, out_dim]],
                        ),
                        in_=out_t[:].rearrange("p (g d) -> p g d", d=out_dim)[
                            :, :ng, :
                        ],
                    )
    nc.compile()
    return nc


_CACHE = {}


def _get_program(meta):
    key = (
        meta["own_pad"], meta["n_blocks"],
        tuple((b, o, d) for b, o, d, _, _ in meta["chunks"]),
    )
    if key not in _CACHE:
        _CACHE[key] = _build_program(
            meta["n_blocks"], meta["chunks"], meta["own_pad"],
        )
    return _CACHE[key]


def run(h, edge_index, W_msg, Ws, Wd, v, trace=False, trace_kwargs=None):
    in_maps, meta = _preprocess(h, edge_index, W_msg, Ws, Wd, v, NCORES)
    nc = _get_program(meta)
    kwargs = {}
    if trace:
        kwargs = dict(trace=True, **(trace_kwargs or {}))
    res = run_bass_kernel_spmd(nc, in_maps, list(range(NCORES)), **kwargs)
    n, own = meta["n"], meta["own"]
    out_dim = res.results[0]["out"].shape[1]
    full = np.zeros((n, out_dim), dtype=np.float32)
    for c in range(NCORES):
        perm = meta["perms"][c]
        full[c * own + perm] = res.results[c]["out"][:own]
    return full, res


def _spot_check(out, h, edge_index, W_msg, Ws, Wd, v, k=128):
    """Exact fp32 reference on k sampled dst nodes; guards against the rare
    corrupted device execution (re-run once if it trips)."""
    h = np.asarray(h, np.float64)
    ei = np.asarray(edge_index)
    n = h.shape[0]
    loops = np.arange(n, dtype=ei.dtype)
    src = np.concatenate([ei[0], loops])
    dst = np.concatenate([ei[1], loops])
    order = np.argsort(dst, kind="stable")
    dst_s, src_s = dst[order], src[order]
    rng = np.random.default_rng(12345)
    nodes = rng.choice(n, size=k, replace=False)
    lo = np.searchsorted(dst_s, nodes, side="left")
    hi = np.searchsorted(dst_s, nodes, side="right")
    Wsm, Wdm, Wmm = (np.asarray(W, np.float64) for W in (Ws, Wd, W_msg))
    vv = np.asarray(v, np.float64)
    bad = 0
    for j, node in enumerate(nodes):
        sj = src_s[lo[j] : hi[j]]
        e = np.tanh(h[node] @ Wsm.T + h[sj] @ Wdm.T) @ vv
        ex = np.exp(e - e.max())
        alpha = ex / ex.sum()
        ref = np.tanh(alpha @ (h[sj] @ Wmm.T))
        if np.abs(ref - out[node]).max() > 0.05:
            bad += 1
    return bad == 0


def kernel(h, edge_index, W_msg, Ws, Wd, v):
    out, _ = run(h, edge_index, W_msg, Ws, Wd, v)
    if not _spot_check(out, h, edge_index, W_msg, Ws, Wd, v):
        out, _ = run(h, edge_index, W_msg, Ws, Wd, v)
    return out
